# revision 7
# baseline (speedup 1.0000x reference)
"""Trainium2 Bass kernel for nn_Attention (dense transformer cross-attention).

Strategy: data-parallel over batch (B=8) -> one batch element per NeuronCore.
Per core, zero on-chip transposes by choosing layouts up front (host
pre-transposes activations/weights, which is free):

  K^T_h [dh=128, M]   = Wk-chunk^T . memory^T   (per head h, bias via ACT)
  Q^T_h [dh=128, Q]   = (scaled Wq)-chunk^T . query^T
  V     [M, D]        = memory . Wv^T + bv      (natural layout)
  S^T   [M, Q]        = K^T_h chunk (stationary) . Q^T_h
  expS  = ACT Exp with per-partition mask bias (-1e30 -> exact 0), bf16
  sum_q = DVE strided pair-add tree + ones-stationary matmul
          (cross-partition sum, broadcast for free)
  1/sum = DVE reciprocal on a compact [128, 8] layout (DRAM-bounce to
          scatter the 1024 q-sums across partitions), then DRAM-bounce
          partition-broadcast back to [128, Q]
  ctx^T_h [dh, Q]     = V-chunks . expS  (PSUM accum over m-chunks),
                        DVE multiply by 1/sum on drain
  out   [Q, D]        = ctx^T (as lhsT, heads = contraction chunks) . Wf^T
                        + bf via DVE, bf16 out (host upcasts)

The whole kernel is software-pipelined around a single head loop: slot h
runs scores/exp of head h, the PV accumulation of head h-2, the softmax
sum of head h-1, and the K/Q projections of head h+1 (V projections fill
slots 0-1), so ACT/DVE softmax work hides under projection matmuls and
the PE never waits on the softmax chain. Every stationary 128x128 weight
tile feeds two 512-wide matmuls (LDWEIGHTS:MATMUL = 1:2) so weight loads
hide in the PE background weight buffer.

Masked memory positions produce exactly-zero softmax weights, so m-chunks
that are fully masked in every batch are skipped entirely (K/V projection,
scores, exp, PV, p0); the host zero-fills those output columns. For the
reference's fixed mask (last quarter masked) this drops 2 of 8 chunks.

Softmax max-subtraction is skipped: scores are O(1) by construction
(0.02-scale weights), exp is computed in f32 on ACT, so this is exact.
"""

import math

import numpy as np
import ml_dtypes

B = 8
Q = 1024
M = 1024
D = 1024
H = 8
DH = 128
KC = 8  # 128-row contraction chunks per 1024
FT = 512
NT = 2

_BF16 = ml_dtypes.bfloat16
_CACHE = {}


def _mtiles(n):
    """Split free dim n into tiles of <= 512."""
    out = []
    o = 0
    while o < n:
        w = min(FT, n - o)
        out.append(slice(o, o + w))
        o += w
    return out


def _build_program(mc):
    """mc = number of live m-chunks (each 128 memory positions)."""
    import concourse.bass as bass
    import concourse.mybir as mybir
    from concourse.tile import TileContext

    import bass_rust

    f32 = mybir.dt.float32
    bf16 = mybir.dt.bfloat16
    Identity = mybir.ActivationFunctionType.Identity
    Exp = mybir.ActivationFunctionType.Exp

    ME = mc * DH  # effective memory length

    def split_sync_waits(nc):
        """The walrus in this container accepts only ONE sync-wait per
        instruction; Tile freely attaches several. Move excess waits onto
        same-engine NOPs spliced immediately before the instruction."""
        for fn in nc.m.functions:
            for bb in fn.blocks:
                out = []
                for inst in bb.instructions:
                    si = inst.sync_info
                    if si is not None and si.on_wait is not None and len(si.on_wait) > 1:
                        waits = list(si.on_wait)
                        si.on_wait = waits[-1:]
                        for j, w in enumerate(waits[:-1]):
                            nop = bass_rust.InstNoOp(
                                name=f"{inst.name}_sw{j}", ins=[], outs=[])
                            nop.engine = inst.engine
                            nop.sync_info = mybir.SyncInfo(on_wait=[w], on_update=[])
                            out.append(nop)
                    out.append(inst)
                bb.instructions = out

    nc = bass.Bass()

    memT = nc.declare_dram_parameter("memT", [D, ME], bf16, isOutput=False)
    qT = nc.declare_dram_parameter("qT", [D, Q], bf16, isOutput=False)
    wkT = nc.declare_dram_parameter("wkT", [D, D], bf16, isOutput=False)
    wvT = nc.declare_dram_parameter("wvT", [D, D], bf16, isOutput=False)
    wqT = nc.declare_dram_parameter("wqT", [D, D], bf16, isOutput=False)
    wfT = nc.declare_dram_parameter("wfT", [D, D], bf16, isOutput=False)
    bk_pp = nc.declare_dram_parameter("bk_pp", [128, H], f32, isOutput=False)
    bq_pp = nc.declare_dram_parameter("bq_pp", [128, H], f32, isOutput=False)
    mb_pp = nc.declare_dram_parameter("mb_pp", [128, mc], f32, isOutput=False)
    bv_bc = nc.declare_dram_parameter("bv_bc", [128, D], f32, isOutput=False)
    bf_bc = nc.declare_dram_parameter("bf_bc", [128, D], f32, isOutput=False)

    wm = nc.declare_dram_parameter("wm", [Q, D], bf16, isOutput=True)
    p0t = nc.declare_dram_parameter("p0t", [ME, Q], bf16, isOutput=True)

    def chunked(dram_ap):
        # [1024, N] DRAM -> [p=128, c=8, N] access pattern
        return dram_ap.rearrange("(c p) n -> p c n", p=128)

    m_tiles = _mtiles(ME)

    with TileContext(nc) as tc:
      with tc.tile_pool(name="const", bufs=1) as const, \
           tc.tile_pool(name="persist", bufs=1) as persist:
        bkt = const.tile([128, H], f32)
        bqt = const.tile([128, H], f32)
        mbt = const.tile([128, mc], f32)
        bvt = const.tile([128, D], f32)
        bft = const.tile([128, D], f32)
        ones128 = const.tile([128, 128], bf16)
        warm = const.tile([128, 1], f32)
        wf_sb = const.tile([128, KC, D], bf16)

        nc.scalar.dma_start(out=bkt[:], in_=bk_pp[:, :])
        nc.scalar.dma_start(out=bqt[:], in_=bq_pp[:, :])
        nc.scalar.dma_start(out=mbt[:], in_=mb_pp[:, :])
        nc.scalar.dma_start(out=bvt[:], in_=bv_bc[:, :])
        nc.scalar.dma_start(out=bft[:], in_=bf_bc[:, :])
        nc.vector.memset(ones128[:], 1.0)
        # pre-load the ACT exp table set before the first real exp
        nc.scalar.activation(warm[:], bkt[:, 0:1], Exp)

        v_sb = persist.tile([128, mc, D], bf16)
        ctx = [persist.tile([128, Q], bf16, name=f"ctx{h}") for h in range(H)]

        with tc.tile_pool(name="attn", bufs=2) as attn, \
             tc.tile_pool(name="kq", bufs=3) as kq, \
             tc.tile_pool(name="attn3", bufs=2) as attn3, \
             tc.tile_pool(name="dramp", bufs=2, space="DRAM") as dramp, \
             tc.tile_pool(name="ppsum", bufs=1, space="PSUM") as ppsum, \
             tc.tile_pool(name="spsum", bufs=2, space="PSUM") as spsum, \
             tc.tile_pool(name="cpsum", bufs=1, space="PSUM") as cpsum:

          k_t = {}
          q_t = {}

          def emit_K(h):
              hs = slice(h * DH, (h + 1) * DH)
              ps = spsum.tile([128, Q], f32, tag="st", name=f"kp{h}")
              for c in range(KC):
                  for ts_ in m_tiles:
                      nc.tensor.matmul(
                          ps[:, ts_], wk_sb[:, c, hs], mem_c[c][:, ts_],
                          start=(c == 0), stop=(c == KC - 1))
              k_t[h] = kq.tile([128, ME], bf16, tag="k", name=f"k{h}")
              nc.scalar.activation(
                  k_t[h][:], ps[:, 0:ME], Identity, bias=bkt[:, h:h + 1])

          def emit_Q(h):
              hs = slice(h * DH, (h + 1) * DH)
              ps = spsum.tile([128, Q], f32, tag="st", name=f"qp{h}")
              for c in range(KC):
                  for t in range(NT):
                      ts_ = slice(t * FT, (t + 1) * FT)
                      nc.tensor.matmul(
                          ps[:, ts_], wq_sb[:, c, hs], qt_sb[:, c, ts_],
                          start=(c == 0), stop=(c == KC - 1))
              q_t[h] = kq.tile([128, Q], bf16, tag="q", name=f"q{h}")
              nc.scalar.activation(
                  q_t[h][:], ps[:], Identity, bias=bqt[:, h:h + 1])

          def emit_V(mcc):
              ms = slice(mcc * DH, (mcc + 1) * DH)
              ps = spsum.tile([128, Q], f32, tag="st", name=f"vp{mcc}")
              for c in range(KC):
                  for t in range(NT):
                      ts_ = slice(t * FT, (t + 1) * FT)
                      nc.tensor.matmul(
                          ps[:, ts_], mem_c[c][:, ms], wv_c[c][:, ts_],
                          start=(c == 0), stop=(c == KC - 1))
              nc.vector.tensor_add(v_sb[:, mcc, :], ps[:], bvt[:])

          def emit_sum_and_bounce(h, exp_sb):
              """Cross-partition+chunk sum of exp -> 1/sum broadcast [128, Q].
              Ones-stationary matmul accumulation: one LDWEIGHTS, 2*mc
              matmuls, and no DVE tree (DVE is 2.3x-slow silicon)."""
              sum_ps = ppsum.tile([128, Q], f32, tag="pp", name=f"sum{h}")
              for c in range(mc):
                  for t in range(NT):
                      ts_ = slice(t * FT, (t + 1) * FT)
                      nc.tensor.matmul(
                          sum_ps[:, ts_], ones128[:], exp_sb[:, c, ts_],
                          start=(c == 0), stop=(c == mc - 1))
              srow = attn.tile([1, Q], f32, tag="srow", bufs=1,
                               name=f"srow{h}")
              nc.vector.tensor_copy(srow[:], sum_ps[0:1, :])
              srow_d = dramp.tile([1, Q], f32, tag="srow_d")
              nc.sync.dma_start(out=srow_d[:, :], in_=srow[:])
              comp = attn.tile([128, Q // 128], f32, tag="comp")
              nc.sync.dma_start(
                  out=comp[:],
                  in_=srow_d[:, :].rearrange("a (p c) -> (a p) c", p=128))
              rcomp = attn.tile([128, Q // 128], f32, tag="rcomp")
              nc.vector.reciprocal(rcomp[:], comp[:])
              rrow_d = dramp.tile([1, Q], f32, tag="rrow_d")
              nc.sync.dma_start(
                  out=rrow_d[:, :].rearrange("a (p c) -> (a p) c", p=128),
                  in_=rcomp[:])
              rb = attn.tile([128, Q], f32, tag="rb", name=f"rb{h}")
              nc.sync.dma_start(
                  out=rb[:], in_=rrow_d[:, :].partition_broadcast(128))
              return rb

          def emit_pv_pair(ph, pexp, cp, cc):
              phs = slice(ph * DH, (ph + 1) * DH)
              for t in range(NT):
                  ts_ = slice(t * FT, (t + 1) * FT)
                  nc.tensor.matmul(
                      cp[:, ts_], v_sb[:, cc, phs], pexp[:, cc, ts_],
                      start=(cc == 0), stop=(cc == mc - 1))

          def emit_drains(ph, cp, rb):
              for t in range(NT):
                  ts_ = slice(t * FT, (t + 1) * FT)
                  nc.vector.tensor_mul(ctx[ph][:, ts_], cp[:, ts_], rb[:, ts_])

          def emit_p0(pexp, rb):
              for c in range(mc):
                  p0_sb = attn3.tile([128, Q], bf16, tag="p0")
                  nc.gpsimd.tensor_mul(p0_sb[:], pexp[:, c, :], rb[:])
                  nc.sync.dma_start(
                      out=p0t[c * DH:(c + 1) * DH, :], in_=p0_sb[:])

          with tc.tile_pool(name="io1", bufs=1) as io1, \
               tc.tile_pool(name="io2", bufs=1) as io2:
            mem_c = [io1.tile([128, ME], bf16, name=f"mem{c}")
                     for c in range(KC)]
            qt_sb = io1.tile([128, KC, Q], bf16)
            wk_sb = io1.tile([128, KC, D], bf16)
            wq_sb = io1.tile([128, KC, D], bf16)
            wv_c = [io2.tile([128, D], bf16, name=f"wv{c}") for c in range(KC)]
            # V inputs first, chunk-interleaved: V projections start as soon
            # as the first (mem, wv) chunk pair lands.
            for c in range(KC):
                nc.sync.dma_start(out=mem_c[c][:], in_=chunked(memT[:, :])[:, c, :])
                nc.sync.dma_start(out=wv_c[c][:], in_=chunked(wvT[:, :])[:, c, :])
            nc.sync.dma_start(out=wk_sb[:], in_=chunked(wkT[:, :]))
            nc.sync.dma_start(out=qt_sb[:], in_=chunked(qT[:, :]))
            nc.sync.dma_start(out=wq_sb[:], in_=chunked(wqT[:, :]))
            # wf is only needed by phase 3; last on the sync ring so it
            # never competes with the critical input stream.
            nc.sync.dma_start(out=wf_sb[:], in_=chunked(wfT[:, :]))

            for mcc in range(mc):
                emit_V(mcc)
            emit_K(0)
            emit_Q(0)

            exps = {}
            rbs = {}
            pending = None  # (h, acc) awaiting sum+bounce
            for h in range(H):
                exp_sb = attn.tile([128, mc, Q], bf16,
                                   tag=("exp0" if h == 0 else "expS"),
                                   bufs=(1 if h == 0 else 3),
                                   name=f"exp{h}")
                exps[h] = exp_sb
                cp = (cpsum.tile([128, Q], f32, tag="cp", name=f"cp{h - 2}")
                      if h >= 2 else None)
                for c in range(mc):
                    cs = slice(c * DH, (c + 1) * DH)
                    st = spsum.tile([128, Q], f32, tag="st")
                    for t in range(NT):
                        ts_ = slice(t * FT, (t + 1) * FT)
                        nc.tensor.matmul(
                            st[:, ts_], k_t[h][:, cs], q_t[h][:, ts_],
                            start=True, stop=True)
                    if pending is not None and c == 2:
                        rbs[pending] = emit_sum_and_bounce(pending, exps[pending])
                        pending = None
                    nc.scalar.activation(
                        exp_sb[:, c, :], st[:], Exp, bias=mbt[:, c:c + 1])
                    if h >= 2:
                        emit_pv_pair(h - 2, exps[h - 2], cp, c)
                if h >= 2:
                    emit_drains(h - 2, cp, rbs[h - 2])
                    if h - 2 == 0:
                        emit_p0(exps[0], rbs[0])
                    else:
                        del exps[h - 2]
                pending = h
                # projection fillers: K/Q of head h+1, V in slots 0-1
                if h + 1 < H:
                    emit_K(h + 1)
                    emit_Q(h + 1)

            # tail: PV(H-2) first (its inputs are long ready) so the PE
            # covers the last head's sum+bounce latency, then PV(H-1).
            cp = cpsum.tile([128, Q], f32, tag="cp", name=f"cp{H - 2}")
            for c in range(mc):
                emit_pv_pair(H - 2, exps[H - 2], cp, c)
            rbs[pending] = emit_sum_and_bounce(pending, exps[pending])
            emit_drains(H - 2, cp, rbs[H - 2])
            cp = cpsum.tile([128, Q], f32, tag="cp", name=f"cp{H - 1}")
            for c in range(mc):
                emit_pv_pair(H - 1, exps[H - 1], cp, c)
            emit_drains(H - 1, cp, rbs[H - 1])

        # ---------------- Phase 3: final projection ----------------
        with tc.tile_pool(name="fin", bufs=3) as fin, \
             tc.tile_pool(name="fpsum", bufs=3, space="PSUM") as fpsum:
            for qc in range(KC):
                qs = slice(qc * DH, (qc + 1) * DH)
                fp = fpsum.tile([128, Q], f32, tag="fp")
                for h in range(H):
                    for t in range(NT):
                        ts_ = slice(t * FT, (t + 1) * FT)
                        nc.tensor.matmul(
                            fp[:, ts_], ctx[h][:, qs], wf_sb[:, h, ts_],
                            start=(h == 0), stop=(h == H - 1))
                of = fin.tile([128, D], bf16, tag="of")
                nc.vector.tensor_add(of[:], fp[:], bft[:])
                eng = nc.scalar if qc % 2 == 0 else nc.sync
                eng.dma_start(out=wm[qs, :], in_=of[:])

    split_sync_waits(nc)
    return nc


def _get_program(mc):
    key = f"mc{mc}"
    if key not in _CACHE:
        _CACHE[key] = _build_program(mc)
    return _CACHE[key]


def _host_prep(query, memory, mask, Wk, bk, Wv, bv, Wq, bq, Wf, bf, live):
    scale = 1.0 / math.sqrt(DH)
    f32 = np.float32
    mc = len(live)
    # memory positions belonging to live chunks, in chunk order
    live_pos = np.concatenate([np.arange(c * DH, (c + 1) * DH) for c in live])

    def t_bf16(a):
        return np.ascontiguousarray(np.asarray(a, dtype=f32).T).astype(_BF16)

    shared = {
        "wkT": t_bf16(Wk),
        "wvT": t_bf16(Wv),
        "wqT": np.ascontiguousarray(
            np.asarray(Wq, dtype=f32).T * f32(scale)).astype(_BF16),
        "wfT": t_bf16(Wf),
        "bk_pp": np.ascontiguousarray(
            np.asarray(bk, dtype=f32).reshape(H, DH).T),
        "bq_pp": np.ascontiguousarray(
            (np.asarray(bq, dtype=f32) * f32(scale)).reshape(H, DH).T),
        "bv_bc": np.ascontiguousarray(
            np.broadcast_to(np.asarray(bv, dtype=f32), (128, D))),
        "bf_bc": np.ascontiguousarray(
            np.broadcast_to(np.asarray(bf, dtype=f32), (128, D))),
    }
    mask = np.asarray(mask)
    in_maps = []
    for b in range(B):
        mb = np.where(mask[b, live_pos], f32(-1e30), f32(0.0)).astype(f32)
        in_maps.append({
            **shared,
            "memT": np.ascontiguousarray(
                np.asarray(memory[b], dtype=f32).T[:, live_pos]).astype(_BF16),
            "qT": t_bf16(query[b]),
            "mb_pp": np.ascontiguousarray(mb.reshape(mc, DH).T),
        })
    return in_maps


def kernel(query, memory, mask, Wk, bk, Wv, bv, Wq, bq, Wf, bf):
    from concourse.bass_utils import run_bass_kernel_spmd

    mask_np = np.asarray(mask)
    # chunks of 128 memory positions that are fully masked in EVERY batch
    # contribute exactly zero to softmax numerator/denominator -> skip them
    chunk_dead = mask_np.reshape(B, KC, DH).all(axis=2).all(axis=0)
    live = [c for c in range(KC) if not chunk_dead[c]]
    mc = len(live)

    nc = _get_program(mc)
    in_maps = _host_prep(query, memory, mask, Wk, bk, Wv, bv, Wq, bq, Wf, bf,
                         live)
    res = run_bass_kernel_spmd(nc, in_maps, core_ids=list(range(B)))
    wm = np.stack([res.results[b]["wm"].astype(np.float32) for b in range(B)])
    w0 = np.zeros((B, Q, M), dtype=np.float32)
    for b in range(B):
        p0 = res.results[b]["p0t"].astype(np.float32)  # [ME, Q]
        for i, c in enumerate(live):
            w0[b, :, c * DH:(c + 1) * DH] = p0[i * DH:(i + 1) * DH, :].T
    return wm, w0


# revision 8
# speedup vs baseline: 1.1272x; 1.1272x over previous
"""Trainium2 Bass kernel for nn_Attention (dense transformer cross-attention).

Strategy: data-parallel over batch (B=8) -> one batch element per NeuronCore.
Per core, zero on-chip transposes by choosing layouts up front (host
pre-transposes activations/weights, which is free):

  K^T_h [dh=128, M]   = Wk-chunk^T . memory^T   (per head h, bias via ACT)
  Q^T_h [dh=128, Q]   = (scaled Wq)-chunk^T . query^T
  V     [M, D]        = memory . Wv^T + bv      (natural layout)
  S^T   [M, Q]        = K^T_h chunk (stationary) . Q^T_h
  expS  = ACT Exp with per-partition mask bias (-1e30 -> exact 0), bf16
  sum_q = DVE strided pair-add tree + ones-stationary matmul
          (cross-partition sum, broadcast for free)
  1/sum = DVE reciprocal on a compact [128, 8] layout (DRAM-bounce to
          scatter the 1024 q-sums across partitions), then DRAM-bounce
          partition-broadcast back to [128, Q]
  ctx^T_h [dh, Q]     = V-chunks . expS  (PSUM accum over m-chunks),
                        DVE multiply by 1/sum on drain
  out   [Q, D]        = ctx^T (as lhsT, heads = contraction chunks) . Wf^T
                        + bf via DVE, bf16 out (host upcasts)

The whole kernel is software-pipelined around a single head loop: slot h
runs scores/exp of head h, the PV accumulation of head h-2, the softmax
sum of head h-1, and the K/Q projections of head h+1 (V projections fill
slots 0-1), so ACT/DVE softmax work hides under projection matmuls and
the PE never waits on the softmax chain. Every stationary 128x128 weight
tile feeds two 512-wide matmuls (LDWEIGHTS:MATMUL = 1:2) so weight loads
hide in the PE background weight buffer.

Masked memory positions produce exactly-zero softmax weights, so m-chunks
that are fully masked in every batch are skipped entirely (K/V projection,
scores, exp, PV, p0); the host zero-fills those output columns. For the
reference's fixed mask (last quarter masked) this drops 2 of 8 chunks.

Softmax max-subtraction is skipped: scores are O(1) by construction
(0.02-scale weights), exp is computed in f32 on ACT, so this is exact.
"""

import math

import numpy as np
import ml_dtypes

B = 8
Q = 1024
M = 1024
D = 1024
H = 8
DH = 128
KC = 8  # 128-row contraction chunks per 1024
FT = 512
NT = 2

_BF16 = ml_dtypes.bfloat16
_CACHE = {}


def _mtiles(n):
    """Split free dim n into tiles of <= 512."""
    out = []
    o = 0
    while o < n:
        w = min(FT, n - o)
        out.append(slice(o, o + w))
        o += w
    return out


def _build_program(mc):
    """mc = number of live m-chunks (each 128 memory positions)."""
    import concourse.bass as bass
    import concourse.mybir as mybir
    from concourse.tile import TileContext

    import bass_rust

    f32 = mybir.dt.float32
    bf16 = mybir.dt.bfloat16
    Identity = mybir.ActivationFunctionType.Identity
    Exp = mybir.ActivationFunctionType.Exp

    ME = mc * DH  # effective memory length

    def split_sync_waits(nc):
        """The walrus in this container accepts only ONE sync-wait per
        instruction; Tile freely attaches several. Move excess waits onto
        same-engine NOPs spliced immediately before the instruction."""
        for fn in nc.m.functions:
            for bb in fn.blocks:
                out = []
                for inst in bb.instructions:
                    si = inst.sync_info
                    if si is not None and si.on_wait is not None and len(si.on_wait) > 1:
                        waits = list(si.on_wait)
                        si.on_wait = waits[-1:]
                        for j, w in enumerate(waits[:-1]):
                            nop = bass_rust.InstNoOp(
                                name=f"{inst.name}_sw{j}", ins=[], outs=[])
                            nop.engine = inst.engine
                            nop.sync_info = mybir.SyncInfo(on_wait=[w], on_update=[])
                            out.append(nop)
                    out.append(inst)
                bb.instructions = out

    nc = bass.Bass()

    memT = nc.declare_dram_parameter("memT", [D, ME], bf16, isOutput=False)
    qT = nc.declare_dram_parameter("qT", [D, Q], bf16, isOutput=False)
    wkT = nc.declare_dram_parameter("wkT", [D, D], bf16, isOutput=False)
    wvT = nc.declare_dram_parameter("wvT", [D, D], bf16, isOutput=False)
    wqT = nc.declare_dram_parameter("wqT", [D, D], bf16, isOutput=False)
    wfT = nc.declare_dram_parameter("wfT", [D, D], bf16, isOutput=False)
    bk_pp = nc.declare_dram_parameter("bk_pp", [128, H], f32, isOutput=False)
    bq_pp = nc.declare_dram_parameter("bq_pp", [128, H], f32, isOutput=False)
    mb_pp = nc.declare_dram_parameter("mb_pp", [128, mc], f32, isOutput=False)
    bv_bc = nc.declare_dram_parameter("bv_bc", [128, D], f32, isOutput=False)
    bf_bc = nc.declare_dram_parameter("bf_bc", [128, D], f32, isOutput=False)

    wm = nc.declare_dram_parameter("wm", [Q, D], bf16, isOutput=True)
    p0t = nc.declare_dram_parameter("p0t", [ME, Q], bf16, isOutput=True)

    def chunked(dram_ap):
        # [1024, N] DRAM -> [p=128, c=8, N] access pattern
        return dram_ap.rearrange("(c p) n -> p c n", p=128)

    m_tiles = _mtiles(ME)

    with TileContext(nc) as tc:
      with tc.tile_pool(name="const", bufs=1) as const, \
           tc.tile_pool(name="persist", bufs=1) as persist:
        bkt = const.tile([128, H], f32)
        bqt = const.tile([128, H], f32)
        mbt = const.tile([128, mc], f32)
        bvt = const.tile([128, D], f32)
        bft = const.tile([128, D], f32)
        ones128 = const.tile([128, 128], bf16)
        warm = const.tile([128, 1], f32)
        wf_sb = const.tile([128, KC, D], bf16)

        nc.scalar.dma_start(out=bkt[:], in_=bk_pp[:, :])
        nc.scalar.dma_start(out=bqt[:], in_=bq_pp[:, :])
        nc.scalar.dma_start(out=mbt[:], in_=mb_pp[:, :])
        nc.scalar.dma_start(out=bvt[:], in_=bv_bc[:, :])
        nc.scalar.dma_start(out=bft[:], in_=bf_bc[:, :])
        nc.vector.memset(ones128[:], 1.0)
        # pre-load the ACT exp table set before the first real exp
        nc.scalar.activation(warm[:], bkt[:, 0:1], Exp)

        v_sb = persist.tile([128, mc, D], bf16)
        ctx = [persist.tile([128, Q], bf16, name=f"ctx{h}") for h in range(H)]

        with tc.tile_pool(name="attn", bufs=2) as attn, \
             tc.tile_pool(name="kq", bufs=3) as kq, \
             tc.tile_pool(name="attn3", bufs=2) as attn3, \
             tc.tile_pool(name="dramp", bufs=2, space="DRAM") as dramp, \
             tc.tile_pool(name="ppsum", bufs=1, space="PSUM") as ppsum, \
             tc.tile_pool(name="spsum", bufs=2, space="PSUM") as spsum, \
             tc.tile_pool(name="cpsum", bufs=1, space="PSUM") as cpsum:

          k_t = {}
          q_t = {}

          def emit_K(h):
              hs = slice(h * DH, (h + 1) * DH)
              ps = ppsum.tile([128, Q], f32, tag="pp", name=f"kp{h}")
              for c in range(KC):
                  for ts_ in m_tiles:
                      nc.tensor.matmul(
                          ps[:, ts_], wk_sb[:, c, hs], mem_c[c][:, ts_],
                          start=(c == 0), stop=(c == KC - 1))
              k_t[h] = kq.tile([128, ME], bf16, tag="k", name=f"k{h}")
              nc.scalar.activation(
                  k_t[h][:], ps[:, 0:ME], Identity, bias=bkt[:, h:h + 1])

          def emit_Q(h):
              hs = slice(h * DH, (h + 1) * DH)
              ps = ppsum.tile([128, Q], f32, tag="pp", name=f"qp{h}")
              for c in range(KC):
                  for t in range(NT):
                      ts_ = slice(t * FT, (t + 1) * FT)
                      nc.tensor.matmul(
                          ps[:, ts_], wq_sb[:, c, hs], qt_sb[:, c, ts_],
                          start=(c == 0), stop=(c == KC - 1))
              q_t[h] = kq.tile([128, Q], bf16, tag="q", name=f"q{h}")
              nc.scalar.activation(
                  q_t[h][:], ps[:], Identity, bias=bqt[:, h:h + 1])

          def emit_V(mcc):
              ms = slice(mcc * DH, (mcc + 1) * DH)
              ps = spsum.tile([128, Q], f32, tag="st", name=f"vp{mcc}")
              for c in range(KC):
                  for t in range(NT):
                      ts_ = slice(t * FT, (t + 1) * FT)
                      nc.tensor.matmul(
                          ps[:, ts_], mem_c[c][:, ms], wv_c[c][:, ts_],
                          start=(c == 0), stop=(c == KC - 1))
              nc.vector.tensor_add(v_sb[:, mcc, :], ps[:], bvt[:])

          def emit_sum_and_bounce(h, exp_sb):
              """Cross-partition+chunk sum of exp -> 1/sum broadcast [128, Q].
              Ones-stationary matmul accumulation: one LDWEIGHTS, 2*mc
              matmuls, and no DVE tree (DVE is 2.3x-slow silicon)."""
              sum_ps = ppsum.tile([128, Q], f32, tag="pp", name=f"sum{h}")
              for c in range(mc):
                  for t in range(NT):
                      ts_ = slice(t * FT, (t + 1) * FT)
                      nc.tensor.matmul(
                          sum_ps[:, ts_], ones128[:], exp_sb[:, c, ts_],
                          start=(c == 0), stop=(c == mc - 1))
              srow = attn.tile([1, Q], f32, tag="srow", bufs=1,
                               name=f"srow{h}")
              nc.vector.tensor_copy(srow[:], sum_ps[0:1, :])
              srow_d = dramp.tile([1, Q], f32, tag="srow_d")
              nc.sync.dma_start(out=srow_d[:, :], in_=srow[:])
              comp = attn.tile([128, Q // 128], f32, tag="comp")
              nc.sync.dma_start(
                  out=comp[:],
                  in_=srow_d[:, :].rearrange("a (p c) -> (a p) c", p=128))
              rcomp = attn.tile([128, Q // 128], f32, tag="rcomp")
              nc.vector.reciprocal(rcomp[:], comp[:])
              rrow_d = dramp.tile([1, Q], f32, tag="rrow_d")
              nc.sync.dma_start(
                  out=rrow_d[:, :].rearrange("a (p c) -> (a p) c", p=128),
                  in_=rcomp[:])
              rb = attn.tile([128, Q], f32, tag="rb", name=f"rb{h}")
              nc.sync.dma_start(
                  out=rb[:], in_=rrow_d[:, :].partition_broadcast(128))
              return rb

          def emit_pv_pair(ph, pexp, cp, cc):
              phs = slice(ph * DH, (ph + 1) * DH)
              for t in range(NT):
                  ts_ = slice(t * FT, (t + 1) * FT)
                  nc.tensor.matmul(
                      cp[:, ts_], v_sb[:, cc, phs], pexp[:, cc, ts_],
                      start=(cc == 0), stop=(cc == mc - 1))

          def emit_drains(ph, cp, rb):
              for t in range(NT):
                  ts_ = slice(t * FT, (t + 1) * FT)
                  nc.vector.tensor_mul(ctx[ph][:, ts_], cp[:, ts_], rb[:, ts_])

          def emit_p0(pexp, rb):
              for c in range(mc):
                  p0_sb = attn3.tile([128, Q], bf16, tag="p0")
                  nc.gpsimd.tensor_mul(p0_sb[:], pexp[:, c, :], rb[:])
                  nc.sync.dma_start(
                      out=p0t[c * DH:(c + 1) * DH, :], in_=p0_sb[:])

          with tc.tile_pool(name="io1", bufs=1) as io1, \
               tc.tile_pool(name="io2", bufs=1) as io2:
            mem_c = [io1.tile([128, ME], bf16, name=f"mem{c}")
                     for c in range(KC)]
            qt_sb = io1.tile([128, KC, Q], bf16)
            wk_sb = io1.tile([128, KC, D], bf16)
            wq_sb = io1.tile([128, KC, D], bf16)
            wv_c = [io2.tile([128, D], bf16, name=f"wv{c}") for c in range(KC)]
            # V inputs first, chunk-interleaved: V projections start as soon
            # as the first (mem, wv) chunk pair lands.
            for c in range(KC):
                nc.sync.dma_start(out=mem_c[c][:], in_=chunked(memT[:, :])[:, c, :])
                nc.sync.dma_start(out=wv_c[c][:], in_=chunked(wvT[:, :])[:, c, :])
            nc.sync.dma_start(out=wk_sb[:], in_=chunked(wkT[:, :]))
            nc.sync.dma_start(out=qt_sb[:], in_=chunked(qT[:, :]))
            nc.sync.dma_start(out=wq_sb[:], in_=chunked(wqT[:, :]))
            # wf is only needed by phase 3; last on the sync ring so it
            # never competes with the critical input stream.
            nc.sync.dma_start(out=wf_sb[:], in_=chunked(wfT[:, :]))

            for mcc in range(mc):
                emit_V(mcc)
            emit_K(0)
            emit_Q(0)

            exps = {}
            rbs = {}
            pending = None  # (h, acc) awaiting sum+bounce
            for h in range(H):
                exp_sb = attn.tile([128, mc, Q], bf16,
                                   tag=("exp0" if h == 0 else "expS"),
                                   bufs=(1 if h == 0 else 3),
                                   name=f"exp{h}")
                exps[h] = exp_sb
                cp = (cpsum.tile([128, Q], f32, tag="cp", name=f"cp{h - 2}")
                      if h >= 2 else None)
                for c in range(mc):
                    cs = slice(c * DH, (c + 1) * DH)
                    st = spsum.tile([128, Q], f32, tag="st")
                    for t in range(NT):
                        ts_ = slice(t * FT, (t + 1) * FT)
                        nc.tensor.matmul(
                            st[:, ts_], k_t[h][:, cs], q_t[h][:, ts_],
                            start=True, stop=True)
                    if pending is not None and c == 2:
                        rbs[pending] = emit_sum_and_bounce(pending, exps[pending])
                        pending = None
                    nc.scalar.activation(
                        exp_sb[:, c, :], st[:], Exp, bias=mbt[:, c:c + 1])
                    if h >= 2:
                        emit_pv_pair(h - 2, exps[h - 2], cp, c)
                if h >= 2:
                    emit_drains(h - 2, cp, rbs[h - 2])
                    if h - 2 == 0:
                        emit_p0(exps[0], rbs[0])
                    else:
                        del exps[h - 2]
                pending = h
                # projection fillers: K/Q of head h+1, V in slots 0-1
                if h + 1 < H:
                    emit_K(h + 1)
                    emit_Q(h + 1)

            # tail: PV(H-2) first (its inputs are long ready) so the PE
            # covers the last head's sum+bounce latency, then PV(H-1).
            cp = cpsum.tile([128, Q], f32, tag="cp", name=f"cp{H - 2}")
            for c in range(mc):
                emit_pv_pair(H - 2, exps[H - 2], cp, c)
            rbs[pending] = emit_sum_and_bounce(pending, exps[pending])
            emit_drains(H - 2, cp, rbs[H - 2])
            cp = cpsum.tile([128, Q], f32, tag="cp", name=f"cp{H - 1}")
            for c in range(mc):
                emit_pv_pair(H - 1, exps[H - 1], cp, c)
            emit_drains(H - 1, cp, rbs[H - 1])

        # ---------------- Phase 3: final projection ----------------
        with tc.tile_pool(name="fin", bufs=3) as fin, \
             tc.tile_pool(name="fpsum", bufs=3, space="PSUM") as fpsum:
            for qc in range(KC):
                qs = slice(qc * DH, (qc + 1) * DH)
                fp = fpsum.tile([128, Q], f32, tag="fp")
                for h in range(H):
                    for t in range(NT):
                        ts_ = slice(t * FT, (t + 1) * FT)
                        nc.tensor.matmul(
                            fp[:, ts_], ctx[h][:, qs], wf_sb[:, h, ts_],
                            start=(h == 0), stop=(h == H - 1))
                of = fin.tile([128, D], bf16, tag="of")
                nc.vector.tensor_add(of[:], fp[:], bft[:])
                eng = nc.scalar if qc % 2 == 0 else nc.sync
                eng.dma_start(out=wm[qs, :], in_=of[:])

    split_sync_waits(nc)
    return nc


def _get_program(mc):
    key = f"mc{mc}"
    if key not in _CACHE:
        _CACHE[key] = _build_program(mc)
    return _CACHE[key]


def _host_prep(query, memory, mask, Wk, bk, Wv, bv, Wq, bq, Wf, bf, live):
    scale = 1.0 / math.sqrt(DH)
    f32 = np.float32
    mc = len(live)
    # memory positions belonging to live chunks, in chunk order
    live_pos = np.concatenate([np.arange(c * DH, (c + 1) * DH) for c in live])

    def t_bf16(a):
        return np.ascontiguousarray(np.asarray(a, dtype=f32).T).astype(_BF16)

    shared = {
        "wkT": t_bf16(Wk),
        "wvT": t_bf16(Wv),
        "wqT": np.ascontiguousarray(
            np.asarray(Wq, dtype=f32).T * f32(scale)).astype(_BF16),
        "wfT": t_bf16(Wf),
        "bk_pp": np.ascontiguousarray(
            np.asarray(bk, dtype=f32).reshape(H, DH).T),
        "bq_pp": np.ascontiguousarray(
            (np.asarray(bq, dtype=f32) * f32(scale)).reshape(H, DH).T),
        "bv_bc": np.ascontiguousarray(
            np.broadcast_to(np.asarray(bv, dtype=f32), (128, D))),
        "bf_bc": np.ascontiguousarray(
            np.broadcast_to(np.asarray(bf, dtype=f32), (128, D))),
    }
    mask = np.asarray(mask)
    in_maps = []
    for b in range(B):
        mb = np.where(mask[b, live_pos], f32(-1e30), f32(0.0)).astype(f32)
        in_maps.append({
            **shared,
            "memT": np.ascontiguousarray(
                np.asarray(memory[b], dtype=f32).T[:, live_pos]).astype(_BF16),
            "qT": t_bf16(query[b]),
            "mb_pp": np.ascontiguousarray(mb.reshape(mc, DH).T),
        })
    return in_maps


def kernel(query, memory, mask, Wk, bk, Wv, bv, Wq, bq, Wf, bf):
    from concourse.bass_utils import run_bass_kernel_spmd

    mask_np = np.asarray(mask)
    # chunks of 128 memory positions that are fully masked in EVERY batch
    # contribute exactly zero to softmax numerator/denominator -> skip them
    chunk_dead = mask_np.reshape(B, KC, DH).all(axis=2).all(axis=0)
    live = [c for c in range(KC) if not chunk_dead[c]]
    mc = len(live)

    nc = _get_program(mc)
    in_maps = _host_prep(query, memory, mask, Wk, bk, Wv, bv, Wq, bq, Wf, bf,
                         live)
    res = run_bass_kernel_spmd(nc, in_maps, core_ids=list(range(B)))
    wm = np.stack([res.results[b]["wm"].astype(np.float32) for b in range(B)])
    w0 = np.zeros((B, Q, M), dtype=np.float32)
    for b in range(B):
        p0 = res.results[b]["p0t"].astype(np.float32)  # [ME, Q]
        for i, c in enumerate(live):
            w0[b, :, c * DH:(c + 1) * DH] = p0[i * DH:(i + 1) * DH, :].T
    return wm, w0
